# revision 9
# baseline (speedup 1.0000x reference)
"""DeepSeek-style block (MLA attention + top-2 MoE) on 8 Trainium2 NeuronCores.

Strategy:
  L1  (head-parallel):   2 attention heads per core. Scores/AV matmuls in f32r.
  L1b (token-parallel):  out-projection + residual, 512 tokens per core.
  L2  (expert-parallel): 1 expert per core, capacity-padded top-2 dispatch.
Host does: layernorms, router logits/top-k, dispatch/combine (0.1% of FLOPs).
"""
import os
import sys
import types

for _p in ("/opt/trn_rl_repo", "/opt/pypackages"):
    if _p not in sys.path:
        sys.path.append(_p)


def _install_ntff_shim():
    """Best-effort: provide antenv.axon_hooks so BASS_TRACE=1 can profile."""
    try:
        try:
            import antenv
        except ImportError:
            antenv = types.ModuleType("antenv")
            sys.modules["antenv"] = antenv
        if "antenv.axon_hooks" in sys.modules:
            return
        mod = types.ModuleType("antenv.axon_hooks")
        _hook = [None]
        mod.set_axon_ntff_profile_hook = lambda h: _hook.__setitem__(0, h)
        mod.get_axon_ntff_profile_hook = lambda: _hook[0]
        sys.modules["antenv.axon_hooks"] = mod
        antenv.axon_hooks = mod
        from trn_agent_boot.trn_boot import _ntff_profile_via_ctypes
        mod.set_axon_ntff_profile_hook(
            _ntff_profile_via_ctypes("/opt/axon/libaxon_pjrt.so"))
    except Exception:
        pass


_install_ntff_shim()

import numpy as np
import concourse.bass as bass  # noqa: F401
import concourse.mybir as mybir
from concourse import bacc
from concourse.tile import TileContext
from concourse import bass_utils

f32 = mybir.dt.float32
f32r = mybir.dt.float32r
AFT = mybir.ActivationFunctionType

B, S, D = 2, 2048, 1024
H, KVH, HD = 16, 4, 64
QL, KVL = 64, 32
E, TOPK, FF = 8, 2, 4096
T = B * S
NCORES = 8
EPS = 1e-5
P = 128

_cache = {}
LAST_EXEC_NS = []  # exec_time_ns of each launch in the most recent kernel() call


def _ln(x, g, b):
    x64 = x.astype(np.float64)
    m = x64.mean(-1, keepdims=True)
    v = ((x64 - m) ** 2).mean(-1, keepdims=True)
    return (((x64 - m) / np.sqrt(v + EPS)) * g + b).astype(np.float32)


def _run(nc, in_maps):
    res = bass_utils.run_bass_kernel_spmd(nc, in_maps, core_ids=list(range(NCORES)))
    LAST_EXEC_NS.append(res.exec_time_ns)
    return res.results


# ---------------------------------------------------------------- L1: attention
def build_l1():
    nc = bacc.Bacc("TRN2", target_bir_lowering=False, debug=False,
                   num_devices=NCORES)
    hT = nc.dram_tensor("hT", (D, T), f32r, kind="ExternalInput").ap()
    law = nc.dram_tensor("law", (D, QL + KVL), f32r, kind="ExternalInput").ap()
    qbw = nc.dram_tensor("qbw", (QL, 128), f32r, kind="ExternalInput").ap()
    kw = nc.dram_tensor("kw", (KVL, HD), f32r, kind="ExternalInput").ap()
    vw = nc.dram_tensor("vw", (KVL, HD), f32r, kind="ExternalInput").ap()
    cosT = nc.dram_tensor("cosT", (32, T), f32, kind="ExternalInput").ap()
    sinT = nc.dram_tensor("sinT", (32, T), f32, kind="ExternalInput").ap()
    oT = nc.dram_tensor("oT", (128, T), f32, kind="ExternalOutput").ap()
    den = nc.dram_tensor("den", (4, S), f32, kind="ExternalOutput").ap()

    NTT = T // 512          # 8 token tiles of 512
    NTB = T // 128          # 32 token blocks of 128
    LW = QL + KVL           # 96

    with TileContext(nc) as tc:
        with (
            tc.tile_pool(name="cn", bufs=1) as cn,
            tc.tile_pool(name="io", bufs=2) as io,
            tc.tile_pool(name="wk", bufs=3) as wk,
            tc.tile_pool(name="psab", bufs=2, space="PSUM") as psab,
            tc.tile_pool(name="pss", bufs=3, space="PSUM") as pss,
            tc.tile_pool(name="pacc", bufs=2, space="PSUM") as pacc,
        ):
            # ---- constants
            law_sb = cn.tile([P, D // P, LW], f32r)
            nc.sync.dma_start(law_sb[:], law.rearrange("(kc p) f -> p kc f", p=P))
            qbw_sb = cn.tile([QL, 128], f32r)
            nc.sync.dma_start(qbw_sb[:], qbw)
            kw_sb = cn.tile([KVL, HD], f32r)
            nc.sync.dma_start(kw_sb[:], kw)
            vw_sb = cn.tile([KVL, HD], f32r)
            nc.sync.dma_start(vw_sb[:], vw)
            cos_sb = cn.tile([32, T], f32)
            nc.sync.dma_start(cos_sb[:], cosT)
            sin_sb = cn.tile([32, T], f32)
            nc.sync.dma_start(sin_sb[:], sinT)
            # causal {0,1} masks for the 4 diagonal sub-blocks
            masks = []
            for d_ in range(4):
                m = cn.tile([P, 512], f32, tag=f"mask{d_}")
                nc.gpsimd.memset(m[:], 1.0)
                nc.gpsimd.affine_select(
                    out=m[:], in_=m[:], compare_op=mybir.AluOpType.is_ge,
                    fill=0.0, base=-128 * d_, pattern=[[1, 512]],
                    channel_multiplier=-1)
                masks.append(m)

            # ---- persistent activations (separate tiles => base partition 0)
            latqT = cn.tile([QL, T], f32r)    # [64, 4096] q-latent, feature-major
            latkvT = cn.tile([KVL, T], f32r)  # [32, 4096] kv-latent
            rq = [cn.tile([HD, T], f32r, tag=f"rq{hl}", name=f"rq{hl}")
                  for hl in range(2)]
            rkT = cn.tile([HD, T], f32r)      # rope-space k^T (1 kv head)
            v_sb = cn.tile([P, NTB, HD + 1], f32r)  # token-major v + ones col
            ones_sb = cn.tile([P, 1], f32)
            nc.vector.memset(ones_sb[:], 1.0)
            nc.vector.tensor_copy(v_sb[:, :, HD:HD + 1],
                                  ones_sb[:, None, :].to_broadcast([P, NTB, 1]))

            # ---- phase A: latents  lat = law^T @ hT
            for tt in range(NTT):
                ht_t = io.tile([P, D // P, 512], f32r, tag="ht")
                nc.sync.dma_start(
                    ht_t[:],
                    hT.rearrange("(kc p) t -> p kc t", p=P)[:, :, tt * 512:(tt + 1) * 512])
                pl_t = psab.tile([P, 512], f32, tag="t")
                pl = pl_t[:LW, :]
                for kc in range(D // P):
                    nc.tensor.matmul(pl[:], law_sb[:, kc, :], ht_t[:, kc, :],
                                     start=(kc == 0), stop=(kc == D // P - 1))
                nc.vector.tensor_copy(latqT[:, tt * 512:(tt + 1) * 512], pl[:QL, :])
                nc.vector.tensor_copy(latkvT[:, tt * 512:(tt + 1) * 512],
                                      pl[QL:LW, :])

            # ---- phase B: q/k projections (feature-major, rope-space) + v
            for tt in range(NTT):
                sl = slice(tt * 512, (tt + 1) * 512)
                pq = psab.tile([128, 512], f32, tag="t")
                nc.tensor.matmul(pq[:], qbw_sb[:], latqT[:, sl],
                                 start=True, stop=True)
                pk_t = psab.tile([P, 512], f32, tag="t")
                pk = pk_t[:HD, :]
                nc.tensor.matmul(pk[:], kw_sb[:], latkvT[:, sl],
                                 start=True, stop=True)
                cs = cos_sb[:, sl]
                sn = sin_sb[:, sl]
                t1 = wk.tile([32, 512], f32, tag="r1")
                t2 = wk.tile([32, 512], f32, tag="r2")
                # q: two heads, partitions hl*64+[0:32)=x1(even), +[32:64)=x2(odd)
                for hl in range(2):
                    x1 = pq[hl * 64:hl * 64 + 32, :]
                    x2 = pq[hl * 64 + 32:hl * 64 + 64, :]
                    nc.vector.tensor_mul(t1[:], x1, cs)
                    nc.vector.tensor_mul(t2[:], x2, sn)
                    nc.vector.tensor_sub(rq[hl][0:32, sl], t1[:], t2[:])
                    nc.vector.tensor_mul(t1[:], x1, sn)
                    nc.vector.tensor_mul(t2[:], x2, cs)
                    nc.vector.tensor_add(rq[hl][32:64, sl], t1[:], t2[:])
                # k: one head
                nc.vector.tensor_mul(t1[:], pk[0:32, :], cs)
                nc.vector.tensor_mul(t2[:], pk[32:64, :], sn)
                nc.vector.tensor_sub(rkT[0:32, sl], t1[:], t2[:])
                nc.vector.tensor_mul(t1[:], pk[0:32, :], sn)
                nc.vector.tensor_mul(t2[:], pk[32:64, :], cs)
                nc.vector.tensor_add(rkT[32:64, sl], t1[:], t2[:])
            for tb in range(NTB):
                pv_t = psab.tile([P, 512], f32, tag="t")
                pv = pv_t[:, :HD]
                nc.tensor.matmul(pv[:], latkvT[:, tb * 128:(tb + 1) * 128], vw_sb[:],
                                 start=True, stop=True)
                nc.vector.tensor_copy(v_sb[:, tb, :HD], pv[:])

            # ---- phase C: causal attention, units = (b, head_local)
            for b in range(B):
                for hl in range(2):
                    for qt in range(4):
                        q_sl = rq[hl][:, b * S + qt * 512: b * S + (qt + 1) * 512]
                        nkb = 4 * (qt + 1)
                        po = pacc.tile([HD + 1, 512], f32, tag="acc")
                        for kb in range(nkb):
                            pscr = pss.tile([P, 512], f32, tag="s")
                            k_sl = rkT[:, b * S + kb * 128: b * S + (kb + 1) * 128]
                            nc.tensor.matmul(pscr[:], k_sl, q_sl, start=True, stop=True)
                            el = wk.tile([P, 512], f32r, tag="el")
                            nc.scalar.activation(el[:], pscr[:], AFT.Exp)
                            if kb >= 4 * qt:
                                nc.vector.tensor_mul(el[:], el[:], masks[kb - 4 * qt][:])
                            nc.tensor.matmul(po[:], v_sb[:, b * 16 + kb, :], el[:],
                                             start=(kb == 0), stop=(kb == nkb - 1))
                        st = wk.tile([HD + 1, 512], f32, tag="st")
                        nc.vector.tensor_copy(st[:], po[:])
                        c0 = b * S + qt * 512
                        nc.sync.dma_start(oT[hl * 64:(hl + 1) * 64, c0:c0 + 512],
                                          st[:HD, :])
                        nc.sync.dma_start(den[2 * b + hl:2 * b + hl + 1,
                                              qt * 512:(qt + 1) * 512],
                                          st[HD:HD + 1, :])
    nc.compile()
    return nc


# ------------------------------------------------------- L1b: out-proj+residual
def build_l1b():
    nc = bacc.Bacc("TRN2", target_bir_lowering=False, debug=False,
                   num_devices=NCORES)
    TP = T // NCORES  # 512 tokens per core
    onT = nc.dram_tensor("onT", (D, TP), f32r, kind="ExternalInput").ap()
    ow = nc.dram_tensor("ow", (D, D), f32r, kind="ExternalInput").ap()
    obbc = nc.dram_tensor("obbc", (P, D), f32, kind="ExternalInput").ap()
    xc = nc.dram_tensor("xc", (TP, D), f32, kind="ExternalInput").ap()
    x2 = nc.dram_tensor("x2", (TP, D), f32, kind="ExternalOutput").ap()

    with TileContext(nc) as tc:
        with (
            tc.tile_pool(name="cn", bufs=1) as cn,
            tc.tile_pool(name="wk", bufs=3) as wk,
            tc.tile_pool(name="ps", bufs=4, space="PSUM") as ps,
        ):
            onT_sb = cn.tile([P, D // P, TP], f32r)
            nc.sync.dma_start(onT_sb[:], onT.rearrange("(kc p) t -> p kc t", p=P))
            ow_sb = cn.tile([P, D // P, D], f32r)
            nc.sync.dma_start(ow_sb[:], ow.rearrange("(kc p) n -> p kc n", p=P))
            ob_sb = cn.tile([P, D], f32)
            nc.sync.dma_start(ob_sb[:], obbc)
            xc_sb = cn.tile([P, TP // P, D], f32)
            nc.sync.dma_start(xc_sb[:], xc.rearrange("(tb p) n -> p tb n", p=P))
            for tb in range(TP // P):
                for nb in range(D // 512):
                    pm = ps.tile([P, 512], f32, tag="pm")
                    for kc in range(D // P):
                        nc.tensor.matmul(
                            pm[:], onT_sb[:, kc, tb * 128:(tb + 1) * 128],
                            ow_sb[:, kc, nb * 512:(nb + 1) * 512],
                            start=(kc == 0), stop=(kc == D // P - 1))
                    y = wk.tile([P, 512], f32, tag="y")
                    nc.vector.tensor_add(y[:], pm[:],
                                         xc_sb[:, tb, nb * 512:(nb + 1) * 512])
                    nc.vector.tensor_add(y[:], y[:], ob_sb[:, nb * 512:(nb + 1) * 512])
                    nc.sync.dma_start(x2[tb * 128:(tb + 1) * 128,
                                         nb * 512:(nb + 1) * 512], y[:])
    nc.compile()
    return nc


# --------------------------------------------------------------- L2: MoE expert
def build_l2(tws):
    C = sum(tws)
    nc = bacc.Bacc("TRN2", target_bir_lowering=False, debug=False,
                   num_devices=NCORES)
    tokT = nc.dram_tensor("tokT", (D, C), f32r, kind="ExternalInput").ap()
    w1 = nc.dram_tensor("w1", (D, FF), f32r, kind="ExternalInput").ap()
    b1 = nc.dram_tensor("b1", (FF,), f32, kind="ExternalInput").ap()
    w2 = nc.dram_tensor("w2", (FF, D), f32r, kind="ExternalInput").ap()
    b2bc = nc.dram_tensor("b2bc", (P, D), f32, kind="ExternalInput").ap()
    gg = nc.dram_tensor("gg", (C,), f32, kind="ExternalInput").ap()
    y = nc.dram_tensor("y", (C, D), f32, kind="ExternalOutput").ap()

    w1r = w1.rearrange("(kc p) f -> p kc f", p=P)
    w2r = w2.rearrange("(fb p) n -> p fb n", p=P)
    NFB = FF // P  # 32

    with TileContext(nc) as tc:
        with (
            tc.tile_pool(name="cn", bufs=1) as cn,
            tc.tile_pool(name="wt", bufs=3) as wt,
            tc.tile_pool(name="hid", bufs=1) as hid,
            tc.tile_pool(name="yo", bufs=3) as yo,
            tc.tile_pool(name="psh", bufs=4, space="PSUM") as psh,
            tc.tile_pool(name="psy", bufs=4, space="PSUM") as psy,
        ):
            tok_sb = cn.tile([P, D // P, C], f32r)
            nc.sync.dma_start(tok_sb[:], tokT.rearrange("(kc p) t -> p kc t", p=P))
            b1_sb = cn.tile([P, NFB], f32)
            nc.sync.dma_start(b1_sb[:], b1.rearrange("(fb p) -> p fb", p=P))
            b2_sb = cn.tile([P, D], f32)
            nc.sync.dma_start(b2_sb[:], b2bc)
            g_sb = cn.tile([P, C // P], f32)
            nc.sync.dma_start(g_sb[:], gg.rearrange("(tb p) -> p tb", p=P))

            t0 = 0
            for tw in tws:
                ht = hid.tile([P, NFB, 512], f32r, tag="hid")
                # fc1 + gelu -> hidden^T
                for fbb in range(8):
                    phs = [psh.tile([P, 512], f32, tag="h", name=f"ph{j}")
                           for j in range(4)]
                    for kc in range(D // P):
                        w1t = wt.tile([P, 512], f32r, tag="w1")
                        nc.sync.dma_start(
                            w1t[:], w1r[:, kc, fbb * 512:(fbb + 1) * 512])
                        for fj in range(4):
                            nc.tensor.matmul(
                                phs[fj][:, :tw],
                                w1t[:, fj * 128:(fj + 1) * 128],
                                tok_sb[:, kc, t0:t0 + tw],
                                start=(kc == 0), stop=(kc == D // P - 1))
                    for fj in range(4):
                        fb = fbb * 4 + fj
                        nc.scalar.activation(ht[:, fb, :tw], phs[fj][:, :tw],
                                             AFT.Gelu, bias=b1_sb[:, fb:fb + 1])
                # fc2 + bias + gate
                for nb in range(D // 512):
                    pys = [psy.tile([P, 512], f32, tag="y", name=f"py{j}")
                           for j in range(tw // P)]
                    for fb in range(NFB):
                        w2t = wt.tile([P, 512], f32r, tag="w2")
                        nc.sync.dma_start(w2t[:], w2r[:, fb, nb * 512:(nb + 1) * 512])
                        for ts_ in range(tw // P):
                            nc.tensor.matmul(
                                pys[ts_][:],
                                ht[:, fb, ts_ * 128:(ts_ + 1) * 128],
                                w2t[:], start=(fb == 0), stop=(fb == NFB - 1))
                    for ts_ in range(tw // P):
                        yt = yo.tile([P, 512], f32, tag="yt")
                        nc.vector.tensor_add(yt[:], pys[ts_][:],
                                             b2_sb[:, nb * 512:(nb + 1) * 512])
                        tglob = t0 // P + ts_
                        nc.vector.tensor_scalar_mul(yt[:], yt[:],
                                                    g_sb[:, tglob:tglob + 1])
                        nc.sync.dma_start(
                            y[t0 + ts_ * 128: t0 + (ts_ + 1) * 128,
                              nb * 512:(nb + 1) * 512], yt[:])
                t0 += tw
    nc.compile()
    return nc


# ------------------------------------------------------------------------ host
def _rope_tables():
    s = np.arange(S, dtype=np.float64)
    inv = 1.0 / (10000.0 ** (np.arange(0, HD, 2, dtype=np.float64) / HD))
    fr = np.outer(s, inv)                      # [S, 32]
    c = np.cos(fr).T.astype(np.float32)        # [32, S]
    sn = np.sin(fr).T.astype(np.float32)
    return (np.ascontiguousarray(np.concatenate([c, c], axis=1)),
            np.ascontiguousarray(np.concatenate([sn, sn], axis=1)))


def kernel(**inputs):
    ins = {k: np.asarray(v) for k, v in inputs.items()}
    x = ins["x"].astype(np.float32, copy=False)
    LAST_EXEC_NS.clear()

    xf = np.ascontiguousarray(x.reshape(T, D))
    h = _ln(xf, ins["ln1_g"], ins["ln1_b"])
    hT = np.ascontiguousarray(h.T)

    # ---------------- L1: attention core
    if "l1" not in _cache:
        _cache["l1"] = build_l1()
    law = np.ascontiguousarray(
        np.concatenate([ins["q_a_w"], ins["kv_a_w"]], axis=1).astype(np.float32))
    cosT, sinT = _rope_tables()
    perm = np.concatenate([np.arange(0, HD, 2), np.arange(1, HD, 2)])
    kvb = ins["kv_b_w"].reshape(KVL, KVH, HD, 2).astype(np.float32)
    in_maps = []
    for c in range(NCORES):
        cols = np.concatenate([(2 * c + hl) * HD + perm for hl in range(2)])
        qbw_c = np.ascontiguousarray(ins["q_b_w"][:, cols].astype(np.float32))
        g = c // 2
        kw_c = np.ascontiguousarray(kvb[:, g, perm, 0] * (HD ** -0.5))
        vw_c = np.ascontiguousarray(kvb[:, g, :, 1])
        in_maps.append({"hT": hT, "law": law, "qbw": qbw_c, "kw": kw_c,
                        "vw": vw_c, "cosT": cosT, "sinT": sinT})
    r1 = _run(_cache["l1"], in_maps)

    oT = np.concatenate([r1[c]["oT"] for c in range(NCORES)], axis=0)  # [1024, T]
    dh = np.empty((H, T), np.float32)
    for c in range(NCORES):
        for b in range(B):
            for hl in range(2):
                dh[2 * c + hl, b * S:(b + 1) * S] = r1[c]["den"][2 * b + hl]
    o_n = (oT.reshape(H, HD, T) / dh[:, None, :]).reshape(D, T)

    # ---------------- L1b: out projection + residual
    if "l1b" not in _cache:
        _cache["l1b"] = build_l1b()
    TP = T // NCORES
    ow = np.ascontiguousarray(ins["out_w"].astype(np.float32))
    obbc = np.ascontiguousarray(
        np.broadcast_to(ins["out_b"].astype(np.float32), (P, D)))
    in_maps = [{"onT": np.ascontiguousarray(o_n[:, c * TP:(c + 1) * TP]),
                "ow": ow, "obbc": obbc,
                "xc": np.ascontiguousarray(xf[c * TP:(c + 1) * TP])}
               for c in range(NCORES)]
    r1b = _run(_cache["l1b"], in_maps)
    x2 = np.concatenate([r1b[c]["x2"] for c in range(NCORES)], axis=0)  # [T, D]

    # ---------------- router (host)
    h2 = _ln(x2, ins["ln2_g"], ins["ln2_b"])
    logits = h2.astype(np.float64) @ ins["router_w"].astype(np.float64)  # [T, E]
    order = np.argsort(-logits, axis=-1, kind="stable")[:, :TOPK]
    tv = np.take_along_axis(logits, order, axis=-1)
    ex = np.exp(tv - tv[:, :1])
    gates2 = ex / ex.sum(-1, keepdims=True)     # [T, 2] float64

    # ---------------- L2: experts
    idx_l, g_l = [], []
    for e in range(E):
        m0 = order[:, 0] == e
        m1 = order[:, 1] == e
        idx_e = np.concatenate([np.nonzero(m0)[0], np.nonzero(m1)[0]])
        gt_e = np.concatenate([gates2[m0, 0], gates2[m1, 1]]).astype(np.float32)
        idx_l.append(idx_e)
        g_l.append(gt_e)
    cap = max(len(i) for i in idx_l)
    C = max(768, -(-cap // 128) * 128)
    tws = tuple([512] * (C // 512) + ([C % 512] if C % 512 else []))
    if ("l2", tws) not in _cache:
        _cache[("l2", tws)] = build_l2(tws)
    h2T = np.ascontiguousarray(h2.T)
    in_maps = []
    for e in range(E):
        n_e = len(idx_l[e])
        tokT_e = np.zeros((D, C), np.float32)
        tokT_e[:, :n_e] = h2T[:, idx_l[e]]
        gfull = np.zeros((C,), np.float32)
        gfull[:n_e] = g_l[e]
        in_maps.append({
            "tokT": tokT_e,
            "w1": np.ascontiguousarray(ins["fc_w"][e].astype(np.float32)),
            "b1": np.ascontiguousarray(ins["fc_b"][e].astype(np.float32)),
            "w2": np.ascontiguousarray(ins["proj_w"][e].astype(np.float32)),
            "b2bc": np.ascontiguousarray(
                np.broadcast_to(ins["proj_b"][e].astype(np.float32), (P, D))),
            "gg": gfull})
    r2 = _run(_cache[("l2", tws)], in_maps)

    moe = np.zeros((T, D), np.float32)
    for e in range(E):
        n_e = len(idx_l[e])
        moe[idx_l[e]] += r2[e]["y"][:n_e]
    moe_ln = _ln(moe, ins["moe_ln_g"], ins["moe_ln_b"])
    out = (x2 + moe_ln).reshape(B, S, D)
    router_logits = logits.astype(np.float32).reshape(B, S, E)
    return out, router_logits


# revision 10
# speedup vs baseline: 1.3551x; 1.3551x over previous
"""DeepSeek-style block (MLA attention + top-2 MoE) on 8 Trainium2 NeuronCores.

Strategy:
  L1  (head-parallel):   2 attention heads per core. Scores/AV matmuls in f32r.
  L1b (token-parallel):  out-projection + residual, 512 tokens per core.
  L2  (expert-parallel): 1 expert per core, capacity-padded top-2 dispatch.
Host does: layernorms, router logits/top-k, dispatch/combine (0.1% of FLOPs).
"""
import os
import sys
import types

for _p in ("/opt/trn_rl_repo", "/opt/pypackages"):
    if _p not in sys.path:
        sys.path.append(_p)


def _install_ntff_shim():
    """Best-effort: provide antenv.axon_hooks so BASS_TRACE=1 can profile."""
    try:
        try:
            import antenv
        except ImportError:
            antenv = types.ModuleType("antenv")
            sys.modules["antenv"] = antenv
        if "antenv.axon_hooks" in sys.modules:
            return
        mod = types.ModuleType("antenv.axon_hooks")
        _hook = [None]
        mod.set_axon_ntff_profile_hook = lambda h: _hook.__setitem__(0, h)
        mod.get_axon_ntff_profile_hook = lambda: _hook[0]
        sys.modules["antenv.axon_hooks"] = mod
        antenv.axon_hooks = mod
        from trn_agent_boot.trn_boot import _ntff_profile_via_ctypes
        mod.set_axon_ntff_profile_hook(
            _ntff_profile_via_ctypes("/opt/axon/libaxon_pjrt.so"))
    except Exception:
        pass


_install_ntff_shim()

import numpy as np
import ml_dtypes
import concourse.bass as bass  # noqa: F401
import concourse.mybir as mybir
from concourse import bacc
from concourse.tile import TileContext
from concourse import bass_utils

f32 = mybir.dt.float32
f32r = mybir.dt.float32r
AFT = mybir.ActivationFunctionType

B, S, D = 2, 2048, 1024
H, KVH, HD = 16, 4, 64
QL, KVL = 64, 32
E, TOPK, FF = 8, 2, 4096
T = B * S
NCORES = 8
EPS = 1e-5
P = 128

_cache = {}
LAST_EXEC_NS = []  # exec_time_ns of each launch in the most recent kernel() call


def _ln(x, g, b):
    x64 = x.astype(np.float64)
    m = x64.mean(-1, keepdims=True)
    v = ((x64 - m) ** 2).mean(-1, keepdims=True)
    return (((x64 - m) / np.sqrt(v + EPS)) * g + b).astype(np.float32)


def _run(nc, in_maps):
    res = bass_utils.run_bass_kernel_spmd(nc, in_maps, core_ids=list(range(NCORES)))
    LAST_EXEC_NS.append(res.exec_time_ns)
    return res.results


# ---------------------------------------------------------------- L1: attention
def build_l1():
    nc = bacc.Bacc("TRN2", target_bir_lowering=False, debug=False,
                   num_devices=NCORES)
    hT = nc.dram_tensor("hT", (D, T), f32r, kind="ExternalInput").ap()
    law = nc.dram_tensor("law", (D, QL + KVL), f32r, kind="ExternalInput").ap()
    qbw = nc.dram_tensor("qbw", (QL, 128), f32r, kind="ExternalInput").ap()
    kw = nc.dram_tensor("kw", (KVL, HD), f32r, kind="ExternalInput").ap()
    vw = nc.dram_tensor("vw", (KVL, HD), f32r, kind="ExternalInput").ap()
    cosT = nc.dram_tensor("cosT", (32, T), f32, kind="ExternalInput").ap()
    sinT = nc.dram_tensor("sinT", (32, T), f32, kind="ExternalInput").ap()
    oT = nc.dram_tensor("oT", (128, T), f32, kind="ExternalOutput").ap()
    den = nc.dram_tensor("den", (4, S), f32, kind="ExternalOutput").ap()

    NTT = T // 512          # 8 token tiles of 512
    NTB = T // 128          # 32 token blocks of 128
    LW = QL + KVL           # 96

    with TileContext(nc) as tc:
        with (
            tc.tile_pool(name="cn", bufs=1) as cn,
            tc.tile_pool(name="io", bufs=2) as io,
            tc.tile_pool(name="wk", bufs=3) as wk,
            tc.tile_pool(name="psab", bufs=2, space="PSUM") as psab,
            tc.tile_pool(name="pss", bufs=2, space="PSUM") as pss,
            tc.tile_pool(name="pacc", bufs=2, space="PSUM") as pacc,
        ):
            # ---- constants
            law_sb = cn.tile([P, D // P, LW], f32r)
            nc.sync.dma_start(law_sb[:], law.rearrange("(kc p) f -> p kc f", p=P))
            qbw_sb = cn.tile([QL, 128], f32r)
            nc.sync.dma_start(qbw_sb[:], qbw)
            kw_sb = cn.tile([KVL, HD], f32r)
            nc.sync.dma_start(kw_sb[:], kw)
            vw_sb = cn.tile([KVL, HD], f32r)
            nc.sync.dma_start(vw_sb[:], vw)
            cos_sb = cn.tile([32, T], f32)
            nc.sync.dma_start(cos_sb[:], cosT)
            sin_sb = cn.tile([32, T], f32)
            nc.sync.dma_start(sin_sb[:], sinT)
            # causal {0,1} wide masks: wm[g] covers diagonal k-block pair (2g, 2g+1)
            wmasks = []
            for g_ in range(2):
                m = cn.tile([P, 1024], f32, tag=f"mask{g_}", name=f"mask{g_}")
                nc.gpsimd.memset(m[:], 1.0)
                for half in range(2):
                    d_ = 2 * g_ + half
                    nc.gpsimd.affine_select(
                        out=m[:, half * 512:(half + 1) * 512],
                        in_=m[:, half * 512:(half + 1) * 512],
                        compare_op=mybir.AluOpType.is_ge,
                        fill=0.0, base=-128 * d_, pattern=[[1, 512]],
                        channel_multiplier=-1)
                wmasks.append(m)

            # ---- persistent activations (separate tiles => base partition 0)
            latqT = cn.tile([QL, T], f32r)    # [64, 4096] q-latent, feature-major
            latkvT = cn.tile([KVL, T], f32r)  # [32, 4096] kv-latent
            rq = [cn.tile([HD, T], f32r, tag=f"rq{hl}", name=f"rq{hl}")
                  for hl in range(2)]
            rkT = cn.tile([HD, T], f32r)      # rope-space k^T (1 kv head)
            v_sb = cn.tile([P, NTB, HD + 1], f32r)  # token-major v + ones col
            ones_sb = cn.tile([P, 1], f32)
            nc.vector.memset(ones_sb[:], 1.0)
            nc.vector.tensor_copy(v_sb[:, :, HD:HD + 1],
                                  ones_sb[:, None, :].to_broadcast([P, NTB, 1]))

            # ---- phase A: latents  lat = law^T @ hT
            for tt in range(NTT):
                ht_t = io.tile([P, D // P, 512], f32r, tag="ht")
                nc.sync.dma_start(
                    ht_t[:],
                    hT.rearrange("(kc p) t -> p kc t", p=P)[:, :, tt * 512:(tt + 1) * 512])
                pl_t = psab.tile([P, 512], f32, tag="t")
                pl = pl_t[:LW, :]
                for kc in range(D // P):
                    nc.tensor.matmul(pl[:], law_sb[:, kc, :], ht_t[:, kc, :],
                                     start=(kc == 0), stop=(kc == D // P - 1))
                nc.vector.tensor_copy(latqT[:, tt * 512:(tt + 1) * 512], pl[:QL, :])
                nc.vector.tensor_copy(latkvT[:, tt * 512:(tt + 1) * 512],
                                      pl[QL:LW, :])

            # ---- phase B: q/k projections (feature-major, rope-space) + v
            for tt in range(NTT):
                sl = slice(tt * 512, (tt + 1) * 512)
                pq = psab.tile([128, 512], f32, tag="t")
                nc.tensor.matmul(pq[:], qbw_sb[:], latqT[:, sl],
                                 start=True, stop=True)
                pk_t = psab.tile([P, 512], f32, tag="t")
                pk = pk_t[:HD, :]
                nc.tensor.matmul(pk[:], kw_sb[:], latkvT[:, sl],
                                 start=True, stop=True)
                cs = cos_sb[:, sl]
                sn = sin_sb[:, sl]
                t1 = wk.tile([32, 512], f32, tag="r1")
                t2 = wk.tile([32, 512], f32, tag="r2")
                # q: two heads, partitions hl*64+[0:32)=x1(even), +[32:64)=x2(odd)
                for hl in range(2):
                    x1 = pq[hl * 64:hl * 64 + 32, :]
                    x2 = pq[hl * 64 + 32:hl * 64 + 64, :]
                    nc.vector.tensor_mul(t1[:], x1, cs)
                    nc.vector.tensor_mul(t2[:], x2, sn)
                    nc.vector.tensor_sub(rq[hl][0:32, sl], t1[:], t2[:])
                    nc.vector.tensor_mul(t1[:], x1, sn)
                    nc.vector.tensor_mul(t2[:], x2, cs)
                    nc.vector.tensor_add(rq[hl][32:64, sl], t1[:], t2[:])
                # k: one head
                nc.vector.tensor_mul(t1[:], pk[0:32, :], cs)
                nc.vector.tensor_mul(t2[:], pk[32:64, :], sn)
                nc.vector.tensor_sub(rkT[0:32, sl], t1[:], t2[:])
                nc.vector.tensor_mul(t1[:], pk[0:32, :], sn)
                nc.vector.tensor_mul(t2[:], pk[32:64, :], cs)
                nc.vector.tensor_add(rkT[32:64, sl], t1[:], t2[:])
            for tb in range(NTB):
                pv_t = psab.tile([P, 512], f32, tag="t")
                pv = pv_t[:, :HD]
                nc.tensor.matmul(pv[:], latkvT[:, tb * 128:(tb + 1) * 128], vw_sb[:],
                                 start=True, stop=True)
                nc.vector.tensor_copy(v_sb[:, tb, :HD], pv[:])

            # ---- phase C: causal attention, units = (b, head_local)
            for b in range(B):
                for hl in range(2):
                    for qt in range(4):
                        q_sl = rq[hl][:, b * S + qt * 512: b * S + (qt + 1) * 512]
                        ngrp = 2 * (qt + 1)
                        po = pacc.tile([HD + 1, 512], f32, tag="acc")
                        for g in range(ngrp):
                            pscr = pss.tile([P, 1024], f32, tag="s")
                            for half in range(2):
                                kb = 2 * g + half
                                k_sl = rkT[:, b * S + kb * 128: b * S + (kb + 1) * 128]
                                nc.tensor.matmul(pscr[:, half * 512:(half + 1) * 512],
                                                 k_sl, q_sl, start=True, stop=True)
                            el = wk.tile([P, 1024], f32r, tag="el")
                            nc.scalar.activation(el[:], pscr[:], AFT.Exp)
                            if g >= 2 * qt:
                                nc.vector.tensor_mul(el[:], el[:], wmasks[g - 2 * qt][:])
                            for half in range(2):
                                kb = 2 * g + half
                                nc.tensor.matmul(po[:], v_sb[:, b * 16 + kb, :],
                                                 el[:, half * 512:(half + 1) * 512],
                                                 start=(kb == 0),
                                                 stop=(kb == 2 * ngrp - 1))
                        st = wk.tile([HD + 1, 512], f32, tag="st")
                        nc.vector.tensor_copy(st[:], po[:])
                        c0 = b * S + qt * 512
                        nc.sync.dma_start(oT[hl * 64:(hl + 1) * 64, c0:c0 + 512],
                                          st[:HD, :])
                        nc.sync.dma_start(den[2 * b + hl:2 * b + hl + 1,
                                              qt * 512:(qt + 1) * 512],
                                          st[HD:HD + 1, :])
    nc.compile()
    return nc


# ------------------------------------------------------- L1b: out-proj+residual
def build_l1b():
    nc = bacc.Bacc("TRN2", target_bir_lowering=False, debug=False,
                   num_devices=NCORES)
    TP = T // NCORES  # 512 tokens per core
    onT = nc.dram_tensor("onT", (D, TP), f32r, kind="ExternalInput").ap()
    ow = nc.dram_tensor("ow", (D, D), f32r, kind="ExternalInput").ap()
    obbc = nc.dram_tensor("obbc", (P, D), f32, kind="ExternalInput").ap()
    xc = nc.dram_tensor("xc", (TP, D), f32, kind="ExternalInput").ap()
    x2 = nc.dram_tensor("x2", (TP, D), f32, kind="ExternalOutput").ap()

    with TileContext(nc) as tc:
        with (
            tc.tile_pool(name="cn", bufs=1) as cn,
            tc.tile_pool(name="wk", bufs=3) as wk,
            tc.tile_pool(name="ps", bufs=4, space="PSUM") as ps,
        ):
            onT_sb = cn.tile([P, D // P, TP], f32r)
            nc.sync.dma_start(onT_sb[:], onT.rearrange("(kc p) t -> p kc t", p=P))
            ow_sb = cn.tile([P, D // P, D], f32r)
            nc.sync.dma_start(ow_sb[:], ow.rearrange("(kc p) n -> p kc n", p=P))
            ob_sb = cn.tile([P, D], f32)
            nc.sync.dma_start(ob_sb[:], obbc)
            xc_sb = cn.tile([P, TP // P, D], f32)
            nc.sync.dma_start(xc_sb[:], xc.rearrange("(tb p) n -> p tb n", p=P))
            for tb in range(TP // P):
                for nb in range(D // 512):
                    pm = ps.tile([P, 512], f32, tag="pm")
                    for kc in range(D // P):
                        nc.tensor.matmul(
                            pm[:], onT_sb[:, kc, tb * 128:(tb + 1) * 128],
                            ow_sb[:, kc, nb * 512:(nb + 1) * 512],
                            start=(kc == 0), stop=(kc == D // P - 1))
                    y = wk.tile([P, 512], f32, tag="y")
                    nc.vector.tensor_add(y[:], pm[:],
                                         xc_sb[:, tb, nb * 512:(nb + 1) * 512])
                    nc.vector.tensor_add(y[:], y[:], ob_sb[:, nb * 512:(nb + 1) * 512])
                    nc.sync.dma_start(x2[tb * 128:(tb + 1) * 128,
                                         nb * 512:(nb + 1) * 512], y[:])
    nc.compile()
    return nc


# --------------------------------------------------------------- L2: MoE expert
def build_l2(tws):
    """bf16 MoE expert FFN: w1 streamed once, w2 resident, hidden resident."""
    C = sum(tws)
    n_tt = len(tws)
    assert n_tt <= 3 and C % P == 0
    bf16 = mybir.dt.bfloat16
    nc = bacc.Bacc("TRN2", target_bir_lowering=False, debug=False,
                   num_devices=NCORES)
    tokT = nc.dram_tensor("tokT", (D, C), bf16, kind="ExternalInput").ap()
    w1 = nc.dram_tensor("w1", (D, FF), bf16, kind="ExternalInput").ap()
    b1 = nc.dram_tensor("b1", (FF,), f32, kind="ExternalInput").ap()
    w2 = nc.dram_tensor("w2", (FF, D), bf16, kind="ExternalInput").ap()
    b2bc = nc.dram_tensor("b2bc", (P, D), f32, kind="ExternalInput").ap()
    gg = nc.dram_tensor("gg", (C,), f32, kind="ExternalInput").ap()
    y = nc.dram_tensor("y", (C, D), f32, kind="ExternalOutput").ap()

    w1r = w1.rearrange("(kc p) f -> p kc f", p=P)
    w2r = w2.rearrange("(fb p) n -> p fb n", p=P)
    NFB = FF // P  # 32
    tts = []
    t0 = 0
    for tw in tws:
        tts.append((t0, tw))
        t0 += tw

    with TileContext(nc) as tc:
        with (
            tc.tile_pool(name="cn", bufs=1) as cn,
            tc.tile_pool(name="wt", bufs=4) as wt,
            tc.tile_pool(name="yo", bufs=3) as yo,
            tc.tile_pool(name="psh", bufs=6, space="PSUM") as psh,
            tc.tile_pool(name="psy", bufs=2, space="PSUM") as psy,
        ):
            tok_sb = cn.tile([P, D // P, C], bf16)
            nc.sync.dma_start(tok_sb[:], tokT.rearrange("(kc p) t -> p kc t", p=P))
            b1_sb = cn.tile([P, NFB], f32)
            nc.sync.dma_start(b1_sb[:], b1.rearrange("(fb p) -> p fb", p=P))
            b2_sb = cn.tile([P, D], f32)
            nc.sync.dma_start(b2_sb[:], b2bc)
            g_sb = cn.tile([P, C // P], f32)
            nc.sync.dma_start(g_sb[:], gg.rearrange("(tb p) -> p tb", p=P))
            w2_sb = cn.tile([P, NFB, D], bf16)
            nc.sync.dma_start(w2_sb[:], w2r)
            ht = cn.tile([P, NFB, C], bf16)

            # fc1 + gelu -> hidden^T (whole C resident)
            for fbb in range(16):          # groups of 2 f1-feature blocks
                phs = [psh.tile([P, 512], f32, tag="h", name=f"ph{j}")
                       for j in range(2 * n_tt)]
                for kc in range(D // P):
                    w1t = wt.tile([P, 256], bf16, tag="w1")
                    nc.sync.dma_start(
                        w1t[:], w1r[:, kc, fbb * 256:(fbb + 1) * 256])
                    for ti, (t0, tw) in enumerate(tts):
                        for fj in range(2):
                            nc.tensor.matmul(
                                phs[ti * 2 + fj][:, :tw],
                                w1t[:, fj * 128:(fj + 1) * 128],
                                tok_sb[:, kc, t0:t0 + tw],
                                start=(kc == 0), stop=(kc == D // P - 1))
                for ti, (t0, tw) in enumerate(tts):
                    for fj in range(2):
                        fb = fbb * 2 + fj
                        nc.scalar.activation(ht[:, fb, t0:t0 + tw],
                                             phs[ti * 2 + fj][:, :tw],
                                             AFT.Gelu, bias=b1_sb[:, fb:fb + 1])
            # fc2 + bias + gate
            for ts_ in range(C // P):
                for nb in range(D // 512):
                    py = psy.tile([P, 512], f32, tag="y")
                    for fb in range(NFB):
                        nc.tensor.matmul(
                            py[:], ht[:, fb, ts_ * 128:(ts_ + 1) * 128],
                            w2_sb[:, fb, nb * 512:(nb + 1) * 512],
                            start=(fb == 0), stop=(fb == NFB - 1))
                    yt = yo.tile([P, 512], f32, tag="yt")
                    nc.vector.tensor_add(yt[:], py[:],
                                         b2_sb[:, nb * 512:(nb + 1) * 512])
                    nc.vector.tensor_scalar_mul(yt[:], yt[:],
                                                g_sb[:, ts_:ts_ + 1])
                    nc.sync.dma_start(
                        y[ts_ * 128:(ts_ + 1) * 128,
                          nb * 512:(nb + 1) * 512], yt[:])
    nc.compile()
    return nc


# ------------------------------------------------------------------------ host
def _rope_tables():
    s = np.arange(S, dtype=np.float64)
    inv = 1.0 / (10000.0 ** (np.arange(0, HD, 2, dtype=np.float64) / HD))
    fr = np.outer(s, inv)                      # [S, 32]
    c = np.cos(fr).T.astype(np.float32)        # [32, S]
    sn = np.sin(fr).T.astype(np.float32)
    return (np.ascontiguousarray(np.concatenate([c, c], axis=1)),
            np.ascontiguousarray(np.concatenate([sn, sn], axis=1)))


def kernel(**inputs):
    ins = {k: np.asarray(v) for k, v in inputs.items()}
    x = ins["x"].astype(np.float32, copy=False)
    LAST_EXEC_NS.clear()

    xf = np.ascontiguousarray(x.reshape(T, D))
    h = _ln(xf, ins["ln1_g"], ins["ln1_b"])
    hT = np.ascontiguousarray(h.T)

    # ---------------- L1: attention core
    if "l1" not in _cache:
        _cache["l1"] = build_l1()
    law = np.ascontiguousarray(
        np.concatenate([ins["q_a_w"], ins["kv_a_w"]], axis=1).astype(np.float32))
    cosT, sinT = _rope_tables()
    perm = np.concatenate([np.arange(0, HD, 2), np.arange(1, HD, 2)])
    kvb = ins["kv_b_w"].reshape(KVL, KVH, HD, 2).astype(np.float32)
    in_maps = []
    for c in range(NCORES):
        cols = np.concatenate([(2 * c + hl) * HD + perm for hl in range(2)])
        qbw_c = np.ascontiguousarray(ins["q_b_w"][:, cols].astype(np.float32))
        g = c // 2
        kw_c = np.ascontiguousarray(kvb[:, g, perm, 0] * (HD ** -0.5))
        vw_c = np.ascontiguousarray(kvb[:, g, :, 1])
        in_maps.append({"hT": hT, "law": law, "qbw": qbw_c, "kw": kw_c,
                        "vw": vw_c, "cosT": cosT, "sinT": sinT})
    r1 = _run(_cache["l1"], in_maps)

    oT = np.concatenate([r1[c]["oT"] for c in range(NCORES)], axis=0)  # [1024, T]
    dh = np.empty((H, T), np.float32)
    for c in range(NCORES):
        for b in range(B):
            for hl in range(2):
                dh[2 * c + hl, b * S:(b + 1) * S] = r1[c]["den"][2 * b + hl]
    o_n = (oT.reshape(H, HD, T) / dh[:, None, :]).reshape(D, T)

    # ---------------- L1b: out projection + residual
    if "l1b" not in _cache:
        _cache["l1b"] = build_l1b()
    TP = T // NCORES
    ow = np.ascontiguousarray(ins["out_w"].astype(np.float32))
    obbc = np.ascontiguousarray(
        np.broadcast_to(ins["out_b"].astype(np.float32), (P, D)))
    in_maps = [{"onT": np.ascontiguousarray(o_n[:, c * TP:(c + 1) * TP]),
                "ow": ow, "obbc": obbc,
                "xc": np.ascontiguousarray(xf[c * TP:(c + 1) * TP])}
               for c in range(NCORES)]
    r1b = _run(_cache["l1b"], in_maps)
    x2 = np.concatenate([r1b[c]["x2"] for c in range(NCORES)], axis=0)  # [T, D]

    # ---------------- router (host)
    h2 = _ln(x2, ins["ln2_g"], ins["ln2_b"])
    logits = h2.astype(np.float64) @ ins["router_w"].astype(np.float64)  # [T, E]
    order = np.argsort(-logits, axis=-1, kind="stable")[:, :TOPK]
    tv = np.take_along_axis(logits, order, axis=-1)
    ex = np.exp(tv - tv[:, :1])
    gates2 = ex / ex.sum(-1, keepdims=True)     # [T, 2] float64

    # ---------------- L2: experts
    idx_l, g_l = [], []
    for e in range(E):
        m0 = order[:, 0] == e
        m1 = order[:, 1] == e
        idx_e = np.concatenate([np.nonzero(m0)[0], np.nonzero(m1)[0]])
        gt_e = np.concatenate([gates2[m0, 0], gates2[m1, 1]]).astype(np.float32)
        idx_l.append(idx_e)
        g_l.append(gt_e)
    cap = max(len(i) for i in idx_l)
    C = max(768, -(-cap // 128) * 128)
    tws = tuple([512] * (C // 512) + ([C % 512] if C % 512 else []))
    if ("l2", tws) not in _cache:
        _cache[("l2", tws)] = build_l2(tws)
    h2T = np.ascontiguousarray(h2.T)
    in_maps = []
    for e in range(E):
        n_e = len(idx_l[e])
        tokT_e = np.zeros((D, C), np.float32)
        tokT_e[:, :n_e] = h2T[:, idx_l[e]]
        gfull = np.zeros((C,), np.float32)
        gfull[:n_e] = g_l[e]
        in_maps.append({
            "tokT": tokT_e.astype(ml_dtypes.bfloat16),
            "w1": np.ascontiguousarray(ins["fc_w"][e].astype(ml_dtypes.bfloat16)),
            "b1": np.ascontiguousarray(ins["fc_b"][e].astype(np.float32)),
            "w2": np.ascontiguousarray(ins["proj_w"][e].astype(ml_dtypes.bfloat16)),
            "b2bc": np.ascontiguousarray(
                np.broadcast_to(ins["proj_b"][e].astype(np.float32), (P, D))),
            "gg": gfull})
    r2 = _run(_cache[("l2", tws)], in_maps)

    moe = np.zeros((T, D), np.float32)
    for e in range(E):
        n_e = len(idx_l[e])
        moe[idx_l[e]] += r2[e]["y"][:n_e]
    moe_ln = _ln(moe, ins["moe_ln_g"], ins["moe_ln_b"])
    out = (x2 + moe_ln).reshape(B, S, D)
    router_logits = logits.astype(np.float32).reshape(B, S, E)
    return out, router_logits


# revision 11
# speedup vs baseline: 1.4895x; 1.0992x over previous
"""DeepSeek-style block (MLA attention + top-2 MoE) on 8 Trainium2 NeuronCores.

Strategy:
  L1  (head-parallel):   2 attention heads per core. Scores/AV matmuls in f32r.
  L1b (token-parallel):  out-projection + residual, 512 tokens per core.
  L2  (expert-parallel): 1 expert per core, capacity-padded top-2 dispatch.
Host does: layernorms, router logits/top-k, dispatch/combine (0.1% of FLOPs).
"""
import os
import sys
import types

for _p in ("/opt/trn_rl_repo", "/opt/pypackages"):
    if _p not in sys.path:
        sys.path.append(_p)


def _install_ntff_shim():
    """Best-effort: provide antenv.axon_hooks so BASS_TRACE=1 can profile."""
    try:
        try:
            import antenv
        except ImportError:
            antenv = types.ModuleType("antenv")
            sys.modules["antenv"] = antenv
        if "antenv.axon_hooks" in sys.modules:
            return
        mod = types.ModuleType("antenv.axon_hooks")
        _hook = [None]
        mod.set_axon_ntff_profile_hook = lambda h: _hook.__setitem__(0, h)
        mod.get_axon_ntff_profile_hook = lambda: _hook[0]
        sys.modules["antenv.axon_hooks"] = mod
        antenv.axon_hooks = mod
        from trn_agent_boot.trn_boot import _ntff_profile_via_ctypes
        mod.set_axon_ntff_profile_hook(
            _ntff_profile_via_ctypes("/opt/axon/libaxon_pjrt.so"))
    except Exception:
        pass


_install_ntff_shim()

import numpy as np
import ml_dtypes
import concourse.bass as bass  # noqa: F401
import concourse.mybir as mybir
from concourse import bacc
from concourse.tile import TileContext
from concourse import bass_utils

f32 = mybir.dt.float32
f32r = mybir.dt.float32r
AFT = mybir.ActivationFunctionType

B, S, D = 2, 2048, 1024
H, KVH, HD = 16, 4, 64
QL, KVL = 64, 32
E, TOPK, FF = 8, 2, 4096
T = B * S
NCORES = 8
EPS = 1e-5
P = 128

_cache = {}
LAST_EXEC_NS = []  # exec_time_ns of each launch in the most recent kernel() call


def _ln(x, g, b):
    x64 = x.astype(np.float64)
    m = x64.mean(-1, keepdims=True)
    v = ((x64 - m) ** 2).mean(-1, keepdims=True)
    return (((x64 - m) / np.sqrt(v + EPS)) * g + b).astype(np.float32)


def _run(nc, in_maps):
    res = bass_utils.run_bass_kernel_spmd(nc, in_maps, core_ids=list(range(NCORES)))
    LAST_EXEC_NS.append(res.exec_time_ns)
    return res.results


# ---------------------------------------------------------------- L1: attention
def build_l1():
    nc = bacc.Bacc("TRN2", target_bir_lowering=False, debug=False,
                   num_devices=NCORES)
    hT = nc.dram_tensor("hT", (D, T), f32r, kind="ExternalInput").ap()
    law = nc.dram_tensor("law", (D, QL + KVL), f32r, kind="ExternalInput").ap()
    qbw = nc.dram_tensor("qbw", (QL, 128), f32r, kind="ExternalInput").ap()
    kw = nc.dram_tensor("kw", (KVL, HD), f32r, kind="ExternalInput").ap()
    vw = nc.dram_tensor("vw", (KVL, HD), f32r, kind="ExternalInput").ap()
    cosT = nc.dram_tensor("cosT", (32, T), f32, kind="ExternalInput").ap()
    sinT = nc.dram_tensor("sinT", (32, T), f32, kind="ExternalInput").ap()
    oT = nc.dram_tensor("oT", (128, T), f32, kind="ExternalOutput").ap()
    den = nc.dram_tensor("den", (4, S), f32, kind="ExternalOutput").ap()

    NTT = T // 512          # 8 token tiles of 512
    NTB = T // 128          # 32 token blocks of 128
    LW = QL + KVL           # 96

    with TileContext(nc) as tc:
        with (
            tc.tile_pool(name="cn", bufs=1) as cn,
            tc.tile_pool(name="io", bufs=2) as io,
            tc.tile_pool(name="wk", bufs=4) as wk,
            tc.tile_pool(name="psab", bufs=2, space="PSUM") as psab,
            tc.tile_pool(name="pss", bufs=4, space="PSUM") as pss,
            tc.tile_pool(name="pacc", bufs=2, space="PSUM") as pacc,
        ):
            # ---- constants
            law_sb = cn.tile([P, D // P, LW], f32r)
            nc.sync.dma_start(law_sb[:], law.rearrange("(kc p) f -> p kc f", p=P))
            qbw_sb = cn.tile([QL, 128], f32r)
            nc.sync.dma_start(qbw_sb[:], qbw)
            kw_sb = cn.tile([KVL, HD], f32r)
            nc.sync.dma_start(kw_sb[:], kw)
            vw_sb = cn.tile([KVL, HD], f32r)
            nc.sync.dma_start(vw_sb[:], vw)
            cos_sb = cn.tile([32, T], f32)
            nc.sync.dma_start(cos_sb[:], cosT)
            sin_sb = cn.tile([32, T], f32)
            nc.sync.dma_start(sin_sb[:], sinT)
            # causal {0,1} masks for the 4 diagonal k-sub-blocks
            masks = []
            for d_ in range(4):
                m = cn.tile([P, 512], f32, tag=f"mask{d_}", name=f"mask{d_}")
                nc.gpsimd.memset(m[:], 1.0)
                nc.gpsimd.affine_select(
                    out=m[:], in_=m[:], compare_op=mybir.AluOpType.is_ge,
                    fill=0.0, base=-128 * d_, pattern=[[1, 512]],
                    channel_multiplier=-1)
                masks.append(m)

            # ---- persistent activations (separate tiles => base partition 0)
            latqT = cn.tile([QL, T], f32r)    # [64, 4096] q-latent, feature-major
            latkvT = cn.tile([KVL, T], f32r)  # [32, 4096] kv-latent
            rq = [cn.tile([HD, T], f32r, tag=f"rq{hl}", name=f"rq{hl}")
                  for hl in range(2)]
            rkT = cn.tile([HD, T], f32r)      # rope-space k^T (1 kv head)
            v_sb = cn.tile([P, NTB, HD + 1], f32r)  # token-major v + ones col
            ones_sb = cn.tile([P, 1], f32)
            nc.vector.memset(ones_sb[:], 1.0)
            nc.vector.tensor_copy(v_sb[:, :, HD:HD + 1],
                                  ones_sb[:, None, :].to_broadcast([P, NTB, 1]))

            # ---- phase A: latents  lat = law^T @ hT
            for tt in range(NTT):
                ht_t = io.tile([P, D // P, 512], f32r, tag="ht")
                nc.sync.dma_start(
                    ht_t[:],
                    hT.rearrange("(kc p) t -> p kc t", p=P)[:, :, tt * 512:(tt + 1) * 512])
                pl_t = psab.tile([P, 512], f32, tag="t")
                pl = pl_t[:LW, :]
                for kc in range(D // P):
                    nc.tensor.matmul(pl[:], law_sb[:, kc, :], ht_t[:, kc, :],
                                     start=(kc == 0), stop=(kc == D // P - 1))
                nc.vector.tensor_copy(latqT[:, tt * 512:(tt + 1) * 512], pl[:QL, :])
                nc.vector.tensor_copy(latkvT[:, tt * 512:(tt + 1) * 512],
                                      pl[QL:LW, :])

            # ---- phase B: q/k projections (feature-major, rope-space) + v
            for tt in range(NTT):
                sl = slice(tt * 512, (tt + 1) * 512)
                pq = psab.tile([128, 512], f32, tag="t")
                nc.tensor.matmul(pq[:], qbw_sb[:], latqT[:, sl],
                                 start=True, stop=True)
                pk_t = psab.tile([P, 512], f32, tag="t")
                pk = pk_t[:HD, :]
                nc.tensor.matmul(pk[:], kw_sb[:], latkvT[:, sl],
                                 start=True, stop=True)
                cs = cos_sb[:, sl]
                sn = sin_sb[:, sl]
                t1 = wk.tile([32, 512], f32, tag="r1")
                t2 = wk.tile([32, 512], f32, tag="r2")
                # q: two heads, partitions hl*64+[0:32)=x1(even), +[32:64)=x2(odd)
                for hl in range(2):
                    x1 = pq[hl * 64:hl * 64 + 32, :]
                    x2 = pq[hl * 64 + 32:hl * 64 + 64, :]
                    nc.vector.tensor_mul(t1[:], x1, cs)
                    nc.vector.tensor_mul(t2[:], x2, sn)
                    nc.vector.tensor_sub(rq[hl][0:32, sl], t1[:], t2[:])
                    nc.vector.tensor_mul(t1[:], x1, sn)
                    nc.vector.tensor_mul(t2[:], x2, cs)
                    nc.vector.tensor_add(rq[hl][32:64, sl], t1[:], t2[:])
                # k: one head
                nc.vector.tensor_mul(t1[:], pk[0:32, :], cs)
                nc.vector.tensor_mul(t2[:], pk[32:64, :], sn)
                nc.vector.tensor_sub(rkT[0:32, sl], t1[:], t2[:])
                nc.vector.tensor_mul(t1[:], pk[0:32, :], sn)
                nc.vector.tensor_mul(t2[:], pk[32:64, :], cs)
                nc.vector.tensor_add(rkT[32:64, sl], t1[:], t2[:])
            for tb in range(NTB):
                pv_t = psab.tile([P, 512], f32, tag="t")
                pv = pv_t[:, :HD]
                nc.tensor.matmul(pv[:], latkvT[:, tb * 128:(tb + 1) * 128], vw_sb[:],
                                 start=True, stop=True)
                nc.vector.tensor_copy(v_sb[:, tb, :HD], pv[:])

            # ---- phase C: causal attention; heads interleaved per k-block
            for b in range(B):
                for qt in range(4):
                    q_sls = [rq[hl][:, b * S + qt * 512: b * S + (qt + 1) * 512]
                             for hl in range(2)]
                    nkb = 4 * (qt + 1)
                    po = [pacc.tile([HD + 1, 512], f32, tag="acc", name=f"po{hl}")
                          for hl in range(2)]
                    for kb in range(nkb):
                        k_sl = rkT[:, b * S + kb * 128: b * S + (kb + 1) * 128]
                        for hl in range(2):
                            pscr = pss.tile([P, 512], f32, tag="s", name=f"ps{hl}")
                            nc.tensor.matmul(pscr[:], k_sl, q_sls[hl],
                                             start=True, stop=True)
                            el = wk.tile([P, 512], f32r, tag=f"el{hl}",
                                         name=f"el{hl}")
                            nc.scalar.activation(el[:], pscr[:], AFT.Exp)
                            if kb >= 4 * qt:
                                nc.vector.tensor_mul(el[:], el[:],
                                                     masks[kb - 4 * qt][:])
                            nc.tensor.matmul(po[hl][:], v_sb[:, b * 16 + kb, :],
                                             el[:], start=(kb == 0),
                                             stop=(kb == nkb - 1))
                    for hl in range(2):
                        st = wk.tile([HD + 1, 512], f32, tag="st")
                        nc.vector.tensor_copy(st[:], po[hl][:])
                        c0 = b * S + qt * 512
                        nc.sync.dma_start(oT[hl * 64:(hl + 1) * 64, c0:c0 + 512],
                                          st[:HD, :])
                        nc.sync.dma_start(den[2 * b + hl:2 * b + hl + 1,
                                              qt * 512:(qt + 1) * 512],
                                          st[HD:HD + 1, :])
    nc.compile()
    return nc


# ------------------------------------------------------- L1b: out-proj+residual
def build_l1b():
    nc = bacc.Bacc("TRN2", target_bir_lowering=False, debug=False,
                   num_devices=NCORES)
    TP = T // NCORES  # 512 tokens per core
    onT = nc.dram_tensor("onT", (D, TP), f32r, kind="ExternalInput").ap()
    ow = nc.dram_tensor("ow", (D, D), f32r, kind="ExternalInput").ap()
    obbc = nc.dram_tensor("obbc", (P, D), f32, kind="ExternalInput").ap()
    xc = nc.dram_tensor("xc", (TP, D), f32, kind="ExternalInput").ap()
    x2 = nc.dram_tensor("x2", (TP, D), f32, kind="ExternalOutput").ap()

    with TileContext(nc) as tc:
        with (
            tc.tile_pool(name="cn", bufs=1) as cn,
            tc.tile_pool(name="wk", bufs=3) as wk,
            tc.tile_pool(name="ps", bufs=4, space="PSUM") as ps,
        ):
            onT_sb = cn.tile([P, D // P, TP], f32r)
            nc.sync.dma_start(onT_sb[:], onT.rearrange("(kc p) t -> p kc t", p=P))
            ow_sb = cn.tile([P, D // P, D], f32r)
            nc.sync.dma_start(ow_sb[:], ow.rearrange("(kc p) n -> p kc n", p=P))
            ob_sb = cn.tile([P, D], f32)
            nc.sync.dma_start(ob_sb[:], obbc)
            xc_sb = cn.tile([P, TP // P, D], f32)
            nc.sync.dma_start(xc_sb[:], xc.rearrange("(tb p) n -> p tb n", p=P))
            for tb in range(TP // P):
                for nb in range(D // 512):
                    pm = ps.tile([P, 512], f32, tag="pm")
                    for kc in range(D // P):
                        nc.tensor.matmul(
                            pm[:], onT_sb[:, kc, tb * 128:(tb + 1) * 128],
                            ow_sb[:, kc, nb * 512:(nb + 1) * 512],
                            start=(kc == 0), stop=(kc == D // P - 1))
                    y = wk.tile([P, 512], f32, tag="y")
                    nc.vector.tensor_add(y[:], pm[:],
                                         xc_sb[:, tb, nb * 512:(nb + 1) * 512])
                    nc.vector.tensor_add(y[:], y[:], ob_sb[:, nb * 512:(nb + 1) * 512])
                    nc.sync.dma_start(x2[tb * 128:(tb + 1) * 128,
                                         nb * 512:(nb + 1) * 512], y[:])
    nc.compile()
    return nc


# --------------------------------------------------------------- L2: MoE expert
def build_l2(tws):
    """bf16 MoE expert FFN: w1 streamed once, w2 resident, hidden resident."""
    C = sum(tws)
    n_tt = len(tws)
    assert n_tt <= 3 and C % P == 0
    bf16 = mybir.dt.bfloat16
    nc = bacc.Bacc("TRN2", target_bir_lowering=False, debug=False,
                   num_devices=NCORES)
    tokT = nc.dram_tensor("tokT", (D, C), bf16, kind="ExternalInput").ap()
    w1 = nc.dram_tensor("w1", (D, FF), bf16, kind="ExternalInput").ap()
    b1 = nc.dram_tensor("b1", (FF,), f32, kind="ExternalInput").ap()
    w2 = nc.dram_tensor("w2", (FF, D), bf16, kind="ExternalInput").ap()
    b2bc = nc.dram_tensor("b2bc", (P, D), f32, kind="ExternalInput").ap()
    gg = nc.dram_tensor("gg", (C,), f32, kind="ExternalInput").ap()
    y = nc.dram_tensor("y", (C, D), f32, kind="ExternalOutput").ap()

    w1r = w1.rearrange("(kc p) f -> p kc f", p=P)
    w2r = w2.rearrange("(fb p) n -> p fb n", p=P)
    NFB = FF // P  # 32
    tts = []
    t0 = 0
    for tw in tws:
        tts.append((t0, tw))
        t0 += tw

    with TileContext(nc) as tc:
        with (
            tc.tile_pool(name="cn", bufs=1) as cn,
            tc.tile_pool(name="wt", bufs=4) as wt,
            tc.tile_pool(name="yo", bufs=3) as yo,
            tc.tile_pool(name="psh", bufs=6, space="PSUM") as psh,
            tc.tile_pool(name="psy", bufs=2, space="PSUM") as psy,
        ):
            tok_sb = cn.tile([P, D // P, C], bf16)
            tokr = tokT.rearrange("(kc p) t -> p kc t", p=P)
            for kc in range(D // P):
                nc.sync.dma_start(tok_sb[:, kc, :], tokr[:, kc, :])
            b1_sb = cn.tile([P, NFB], f32)
            nc.sync.dma_start(b1_sb[:], b1.rearrange("(fb p) -> p fb", p=P))
            b2_sb = cn.tile([P, D], f32)
            nc.sync.dma_start(b2_sb[:], b2bc)
            g_sb = cn.tile([P, C // P], f32)
            nc.sync.dma_start(g_sb[:], gg.rearrange("(tb p) -> p tb", p=P))
            w2_sb = cn.tile([P, NFB, D], bf16)
            nc.sync.dma_start(w2_sb[:], w2r)
            ht = cn.tile([P, NFB, C], bf16)

            # fc1 + gelu -> hidden^T (whole C resident)
            for fbb in range(16):          # groups of 2 f1-feature blocks
                phs = [psh.tile([P, 512], f32, tag="h", name=f"ph{j}")
                       for j in range(2 * n_tt)]
                for kc in range(D // P):
                    w1t = wt.tile([P, 256], bf16, tag="w1")
                    nc.sync.dma_start(
                        w1t[:], w1r[:, kc, fbb * 256:(fbb + 1) * 256])
                    for ti, (t0, tw) in enumerate(tts):
                        for fj in range(2):
                            nc.tensor.matmul(
                                phs[ti * 2 + fj][:, :tw],
                                w1t[:, fj * 128:(fj + 1) * 128],
                                tok_sb[:, kc, t0:t0 + tw],
                                start=(kc == 0), stop=(kc == D // P - 1))
                for ti, (t0, tw) in enumerate(tts):
                    for fj in range(2):
                        fb = fbb * 2 + fj
                        nc.scalar.activation(ht[:, fb, t0:t0 + tw],
                                             phs[ti * 2 + fj][:, :tw],
                                             AFT.Gelu, bias=b1_sb[:, fb:fb + 1])
            # fc2 + bias + gate
            for ts_ in range(C // P):
                for nb in range(D // 512):
                    py = psy.tile([P, 512], f32, tag="y")
                    for fb in range(NFB):
                        nc.tensor.matmul(
                            py[:], ht[:, fb, ts_ * 128:(ts_ + 1) * 128],
                            w2_sb[:, fb, nb * 512:(nb + 1) * 512],
                            start=(fb == 0), stop=(fb == NFB - 1))
                    yt = yo.tile([P, 512], f32, tag="yt")
                    nc.vector.tensor_add(yt[:], py[:],
                                         b2_sb[:, nb * 512:(nb + 1) * 512])
                    nc.vector.tensor_scalar_mul(yt[:], yt[:],
                                                g_sb[:, ts_:ts_ + 1])
                    nc.sync.dma_start(
                        y[ts_ * 128:(ts_ + 1) * 128,
                          nb * 512:(nb + 1) * 512], yt[:])
    nc.compile()
    return nc


# ------------------------------------------------------------------------ host
def _rope_tables():
    s = np.arange(S, dtype=np.float64)
    inv = 1.0 / (10000.0 ** (np.arange(0, HD, 2, dtype=np.float64) / HD))
    fr = np.outer(s, inv)                      # [S, 32]
    c = np.cos(fr).T.astype(np.float32)        # [32, S]
    sn = np.sin(fr).T.astype(np.float32)
    return (np.ascontiguousarray(np.concatenate([c, c], axis=1)),
            np.ascontiguousarray(np.concatenate([sn, sn], axis=1)))


def kernel(**inputs):
    ins = {k: np.asarray(v) for k, v in inputs.items()}
    x = ins["x"].astype(np.float32, copy=False)
    LAST_EXEC_NS.clear()

    xf = np.ascontiguousarray(x.reshape(T, D))
    h = _ln(xf, ins["ln1_g"], ins["ln1_b"])
    hT = np.ascontiguousarray(h.T)

    # ---------------- L1: attention core
    if "l1" not in _cache:
        _cache["l1"] = build_l1()
    law = np.ascontiguousarray(
        np.concatenate([ins["q_a_w"], ins["kv_a_w"]], axis=1).astype(np.float32))
    cosT, sinT = _rope_tables()
    perm = np.concatenate([np.arange(0, HD, 2), np.arange(1, HD, 2)])
    kvb = ins["kv_b_w"].reshape(KVL, KVH, HD, 2).astype(np.float32)
    in_maps = []
    for c in range(NCORES):
        cols = np.concatenate([(2 * c + hl) * HD + perm for hl in range(2)])
        qbw_c = np.ascontiguousarray(ins["q_b_w"][:, cols].astype(np.float32))
        g = c // 2
        kw_c = np.ascontiguousarray(kvb[:, g, perm, 0] * (HD ** -0.5))
        vw_c = np.ascontiguousarray(kvb[:, g, :, 1])
        in_maps.append({"hT": hT, "law": law, "qbw": qbw_c, "kw": kw_c,
                        "vw": vw_c, "cosT": cosT, "sinT": sinT})
    r1 = _run(_cache["l1"], in_maps)

    oT = np.concatenate([r1[c]["oT"] for c in range(NCORES)], axis=0)  # [1024, T]
    dh = np.empty((H, T), np.float32)
    for c in range(NCORES):
        for b in range(B):
            for hl in range(2):
                dh[2 * c + hl, b * S:(b + 1) * S] = r1[c]["den"][2 * b + hl]
    o_n = (oT.reshape(H, HD, T) / dh[:, None, :]).reshape(D, T)

    # ---------------- L1b: out projection + residual
    if "l1b" not in _cache:
        _cache["l1b"] = build_l1b()
    TP = T // NCORES
    ow = np.ascontiguousarray(ins["out_w"].astype(np.float32))
    obbc = np.ascontiguousarray(
        np.broadcast_to(ins["out_b"].astype(np.float32), (P, D)))
    in_maps = [{"onT": np.ascontiguousarray(o_n[:, c * TP:(c + 1) * TP]),
                "ow": ow, "obbc": obbc,
                "xc": np.ascontiguousarray(xf[c * TP:(c + 1) * TP])}
               for c in range(NCORES)]
    r1b = _run(_cache["l1b"], in_maps)
    x2 = np.concatenate([r1b[c]["x2"] for c in range(NCORES)], axis=0)  # [T, D]

    # ---------------- router (host)
    h2 = _ln(x2, ins["ln2_g"], ins["ln2_b"])
    logits = h2.astype(np.float64) @ ins["router_w"].astype(np.float64)  # [T, E]
    order = np.argsort(-logits, axis=-1, kind="stable")[:, :TOPK]
    tv = np.take_along_axis(logits, order, axis=-1)
    ex = np.exp(tv - tv[:, :1])
    gates2 = ex / ex.sum(-1, keepdims=True)     # [T, 2] float64

    # ---------------- L2: experts
    idx_l, g_l = [], []
    for e in range(E):
        m0 = order[:, 0] == e
        m1 = order[:, 1] == e
        idx_e = np.concatenate([np.nonzero(m0)[0], np.nonzero(m1)[0]])
        gt_e = np.concatenate([gates2[m0, 0], gates2[m1, 1]]).astype(np.float32)
        idx_l.append(idx_e)
        g_l.append(gt_e)
    cap = max(len(i) for i in idx_l)
    C = max(768, -(-cap // 128) * 128)
    tws = tuple([512] * (C // 512) + ([C % 512] if C % 512 else []))
    if ("l2", tws) not in _cache:
        _cache[("l2", tws)] = build_l2(tws)
    h2T = np.ascontiguousarray(h2.T)
    in_maps = []
    for e in range(E):
        n_e = len(idx_l[e])
        tokT_e = np.zeros((D, C), np.float32)
        tokT_e[:, :n_e] = h2T[:, idx_l[e]]
        gfull = np.zeros((C,), np.float32)
        gfull[:n_e] = g_l[e]
        in_maps.append({
            "tokT": tokT_e.astype(ml_dtypes.bfloat16),
            "w1": np.ascontiguousarray(ins["fc_w"][e].astype(ml_dtypes.bfloat16)),
            "b1": np.ascontiguousarray(ins["fc_b"][e].astype(np.float32)),
            "w2": np.ascontiguousarray(ins["proj_w"][e].astype(ml_dtypes.bfloat16)),
            "b2bc": np.ascontiguousarray(
                np.broadcast_to(ins["proj_b"][e].astype(np.float32), (P, D))),
            "gg": gfull})
    r2 = _run(_cache[("l2", tws)], in_maps)

    moe = np.zeros((T, D), np.float32)
    for e in range(E):
        n_e = len(idx_l[e])
        moe[idx_l[e]] += r2[e]["y"][:n_e]
    moe_ln = _ln(moe, ins["moe_ln_g"], ins["moe_ln_b"])
    out = (x2 + moe_ln).reshape(B, S, D)
    router_logits = logits.astype(np.float32).reshape(B, S, E)
    return out, router_logits
